# revision 11
# baseline (speedup 1.0000x reference)
"""Trainium2 Bass kernel for nn_DocREModel_Triangle (DocRE block-bilinear model).

Strategy (8 NeuronCores, single SPMD NEFF):
  Phase 1 (pair-parallel): core c owns batch b=c//4 and entity rows
  i in [6*(c%4), 6*(c%4)+6) -> 144 (i,j) pairs. Host prep gathers the
  mention rows of sequence_output (for the logsumexp entity pooling)
  and pre-sums the attention rows over the 4 mentions (the /4 mean
  cancels in the normalization), so the device starts from e_att^T
  [l, ent, h] directly. Device computes pairwise head-products, context
  vectors rs^T (the per-pair 1/sum normalization is folded into the
  PSUM->SBUF copy of rs^T, so the rs matmuls don't wait on the
  reciprocal), and the two tanh extractors hs^T/ts^T [768, 144] (bf16),
  all in a transposed layout (feature dim on partitions, pairs free).
  Collectives (single stream, pipelined against compute): hs extractor
  runs first and an AllToAll redistributes hs^T by s-slices; the ts
  extractor follows with its AllGather split in two halves so AG1 can
  start while Et 3-5 still compute.
  Phase 2 (contraction-parallel): core c holds Wp rows (k, s in
  [8c,8c+8), t) resident in SBUF (bf16, 9.4 MB), with rows retiled so
  each 128-row tile pairs (k=2u, s=j) on partitions 0-63 with
  (k=2u+1, s=j) on partitions 64-127 -- that makes the ts operand of
  the bilinear product a direct slice of the AllGather result in SBUF
  (no replication DMA). hs rows are emitted in (j-dest, kb, s, u) order
  (host-side column permutation of Wh) so the whole AllToAll result
  loads into SBUF with 16 clean DMAs. A selector matmul broadcasts the
  two hs values over the 64 t-positions, emitted two tiles ahead of the
  6 Wp-contraction matmuls so the tensor queue never stalls on the DVE
  multiply; 4 pair-chunks of 288, then the classifier. Bias (bc/8) and
  the self-pair mask are applied before the per-chunk logits AllReduce,
  so the post-AllReduce step is a pure DRAM->DRAM copy to the output.
"""

import numpy as np
import ml_dtypes

bf16 = ml_dtypes.bfloat16

B, L, H, NH = 2, 512, 768, 12
NE, NM = 24, 4
E, BS, C = 768, 64, 97
K = E // BS                      # 12 blocks
NCORE = 8
IPC = NE // 4                    # 6 i-rows per core (4 cores per batch elem)
PL = IPC * NE                    # 144 local pairs
NP = B * NE * NE                 # 1152 global pairs
SL = 64 // NCORE                 # 8 s-values per core
KST = K * SL * BS                # 6144 contraction rows per core
NKT = KST // 128                 # 48 contraction tiles
NU = K // 2                      # 6 k-pairs (u)
NCHUNK = 4                       # pair chunks of 288 (= 2 pair-blocks)
CW = NP // NCHUNK                # 288

# hs row order: e' = uh*384 + j*48 + kb*24 + s_l*3 + u'  (u = 3*uh + u')
# <- e = (2u+kb)*64 + 8j + s_l.  uh outermost lets the AllToAll run as two
# halves pipelined with the hs extractor; j = destination core of each A2A
# slice; the (kb, s, u') nesting gives phase 2 single-stride loads
_PERM2 = np.zeros(E, dtype=np.int64)
for _uh in range(2):
    for _j in range(8):
        for _kb in range(2):
            for _s in range(8):
                for _up in range(3):
                    _PERM2[_uh * 384 + _j * 48 + _kb * 24 + _s * 3 + _up] = (
                        2 * (3 * _uh + _up) + _kb) * 64 + 8 * _j + _s


def _host_prep(inputs):
    """Build the 8 per-core input maps from the full inputs."""
    seq = np.ascontiguousarray(inputs["sequence_output"], dtype=np.float32)
    att = np.ascontiguousarray(inputs["attention"], dtype=np.float32)
    Wh = np.asarray(inputs["Wh"], dtype=np.float32)
    bh = np.asarray(inputs["bh"], dtype=np.float32)
    Wt = np.asarray(inputs["Wt"], dtype=np.float32)
    bt = np.asarray(inputs["bt"], dtype=np.float32)
    Wp = np.asarray(inputs["Wp"], dtype=np.float32)
    Wc = np.asarray(inputs["Wc"], dtype=np.float32)
    bc = np.asarray(inputs["bc"], dtype=np.float32)
    mpos = np.asarray(inputs["mention_pos"]).astype(np.int64)

    wh1p = np.ascontiguousarray(Wh[:H][:, _PERM2].astype(bf16))
    wh2p = np.ascontiguousarray(Wh[H:][:, _PERM2].astype(bf16))
    wt1 = np.ascontiguousarray(Wt[:H].astype(bf16))
    wt2 = np.ascontiguousarray(Wt[H:].astype(bf16))
    bh_p = np.ascontiguousarray(bh[_PERM2].reshape(6, 128).T.astype(np.float32))
    bt_t = np.ascontiguousarray(bt.reshape(6, 128).T.astype(np.float32))
    wc_bf = np.ascontiguousarray(Wc.astype(bf16))
    bc8 = np.ascontiguousarray((bc / NCORE).reshape(C, 1).astype(np.float32))
    wp4 = Wp.reshape(K, 64, BS, H)

    in_maps = []
    for c in range(NCORE):
        b = c // 4
        i0 = (c % 4) * IPC
        ents = list(range(NE)) + list(range(i0, i0 + IPC))  # 24 j-side + 6 i-side

        # host-gathered mention rows of seq: 4 m-blocks at 32-partition alignment
        rows = seq[b][mpos[b, ents, :]]                   # [30, NM, H]
        ment = np.zeros((128, H), np.float32)
        ment.reshape(NM, 32, H)[:, :30] = rows.transpose(1, 0, 2)

        # host mention-sum of attention rows -> e_att^T [L, 30, NH]
        A = att[b][:, mpos[b, ents, :], :]                # [NH, 30, NM, L]
        e_attT = np.ascontiguousarray(A.sum(2).transpose(2, 1, 0).astype(bf16))

        # Wp rows for this core's s-slice, retiled so 128-row tile (u*8+j)
        # is [(k=2u, s=8c+j, t 0..63), (k=2u+1, s=8c+j, t 0..63)]
        wpc = wp4[:, SL * c : SL * (c + 1)]               # [K, 8, 64, H]
        wp_sl = np.ascontiguousarray(
            wpc.reshape(NU, 2, SL, BS, H).transpose(0, 2, 1, 3, 4)
            .reshape(KST, H).astype(bf16)
        )

        in_maps.append(
            {
                "ment": np.ascontiguousarray(ment),
                "e_attT": e_attT,
                "seq_bf": np.ascontiguousarray(seq[b].astype(bf16)),
                "wh1p": wh1p,
                "wh2p": wh2p,
                "wt1": wt1,
                "wt2": wt2,
                "bh_p": bh_p,
                "bt_t": bt_t,
                "wp_sl": wp_sl,
                "wc_bf": wc_bf,
                "bc8": bc8,
            }
        )
    return in_maps


def _build_consts():
    # S2 [128, 30]: sums the 4 mention-row exps per entity during the
    # logsumexp transpose-matmul
    S2 = np.zeros((128, 30), dtype=bf16)
    for m in range(NM):
        for e_i in range(30):
            S2[m * 32 + e_i, e_i] = 1.0
    ones_bf = np.ones((128, 1), dtype=bf16)
    ones_row = np.ones((1, 128), dtype=np.float32)
    # sel4 [16, 8, 128]: broadcasts hs_all partition j (k even) to
    # partitions 0-63 and partition 8+j (k odd) to partitions 64-127
    sel4 = np.zeros((16, 8, 128), dtype=bf16)
    for j in range(8):
        sel4[j, j, 0:64] = 1
        sel4[8 + j, j, 64:128] = 1
    return S2, ones_bf, ones_row, sel4


def _self_cols(ck):
    """Local column indices of self-pairs (i==j) within chunk ck."""
    cols = []
    for d in range(2):
        c = 2 * ck + d
        for il in range(IPC):
            ig = (c % 4) * IPC + il
            cols.append(d * PL + il * NE + ig)
    return cols


def build_bass():
    import concourse.bass as bass
    import concourse.mybir as mybir
    import concourse.tile as tile
    from concourse.bacc import Bacc

    f32 = mybir.dt.float32
    bft = mybir.dt.bfloat16
    AF = mybir.ActivationFunctionType
    ALU = mybir.AluOpType

    nc = Bacc("TRN2", num_devices=NCORE)

    # ---- I/O ----
    ment_dr = nc.dram_tensor("ment", [128, H], f32, kind="ExternalInput")
    eat_dr = nc.dram_tensor("e_attT", [L, 30, NH], bft, kind="ExternalInput")
    seq_dr = nc.dram_tensor("seq_bf", [L, H], bft, kind="ExternalInput")
    wh1_dr = nc.dram_tensor("wh1p", [H, E], bft, kind="ExternalInput")
    wh2_dr = nc.dram_tensor("wh2p", [H, E], bft, kind="ExternalInput")
    wt1_dr = nc.dram_tensor("wt1", [H, E], bft, kind="ExternalInput")
    wt2_dr = nc.dram_tensor("wt2", [H, E], bft, kind="ExternalInput")
    bh_dr = nc.dram_tensor("bh_p", [128, 6], f32, kind="ExternalInput")
    bt_dr = nc.dram_tensor("bt_t", [128, 6], f32, kind="ExternalInput")
    wp_dr = nc.dram_tensor("wp_sl", [KST, H], bft, kind="ExternalInput")
    wc_dr = nc.dram_tensor("wc_bf", [H, C], bft, kind="ExternalInput")
    bc_dr = nc.dram_tensor("bc8", [C, 1], f32, kind="ExternalInput")
    out_lgT = nc.dram_tensor("out_lgT", [C, NP], f32, kind="ExternalOutput")

    S2_np, ones_np, onesrow_np, sel4_np = _build_consts()
    S2_dr = nc.inline_tensor(S2_np, "s2_const")
    ones_dr = nc.inline_tensor(ones_np, "ones_const")
    onesrow_dr = nc.inline_tensor(onesrow_np, "onesrow_const")
    sel4_dr = nc.inline_tensor(sel4_np, "sel4_const")

    # collective buffers
    hs_cc_in = nc.dram_tensor("hs_cc_in", [E, PL], bft)
    hs_cc_out = nc.dram_tensor("hs_cc_out", [E, PL], bft)
    ts_cc_in = nc.dram_tensor("ts_cc_in", [E, PL], bft)
    ts_cc_o1 = nc.dram_tensor("ts_cc_o1", [NCORE, E // 2, PL], bft, addr_space="Shared")
    ts_cc_o2 = nc.dram_tensor("ts_cc_o2", [NCORE, E // 2, PL], bft, addr_space="Shared")
    lg_cc_in = nc.dram_tensor("lg_cc_in", [NCHUNK, C, CW], f32)
    lg_cc_out = nc.dram_tensor("lg_cc_out", [NCHUNK, C, CW], f32, addr_space="Shared")
    groups = [list(range(NCORE))]

    with tile.TileContext(nc) as tc:
        with (
            tc.tile_pool(name="gpool", bufs=1) as gpool,
            tc.tile_pool(name="persist", bufs=1) as persist,
        ):
            # ---------- whole-kernel-lifetime weights / constants ----------
            wp_sb = gpool.tile([128, NKT, H], bft)
            wc_sb = gpool.tile([128, 6, C], bft)
            bc_sb = gpool.tile([C, 1], f32)
            sel4_sb = gpool.tile([16, 8, 128], bft)
            ts_all = gpool.tile([128, NU, NCORE, PL], bft)
            hs_all = gpool.tile([16, NU, NCORE, PL], bft)

            hs_sb = persist.tile([128, 6, PL], bft)
            ts_sb = persist.tile([128, 6, PL], bft)

            with (
                tc.tile_pool(name="p1", bufs=1) as p1,
                tc.tile_pool(name="ps1", bufs=2, space="PSUM") as ps1,
            ):
                # ---------- DMA schedule ----------
                # sync queue: ment + e_att (critical), then the big Wp load
                ment_sb = p1.tile([128, H], f32)
                nc.sync.dma_start(out=ment_sb, in_=ment_dr[:])
                eaT = p1.tile([128, 4, 30, NH], bft)
                for lt in range(4):
                    nc.sync.dma_start(out=eaT[:, lt, :, :],
                                      in_=eat_dr[lt * 128 : (lt + 1) * 128])
                for q in range(4):
                    r0 = q * (KST // 4)
                    nc.sync.dma_start(
                        out=wp_sb[:, q * (NKT // 4) : (q + 1) * (NKT // 4), :],
                        in_=wp_dr[r0 : r0 + KST // 4].rearrange("(a p) h -> p a h", p=128),
                    )
                nc.sync.dma_start(out=wc_sb, in_=wc_dr[:].rearrange("(a p) c -> p a c", p=128))

                # gpsimd queue: small consts then phase-1 h-side weights
                nc.gpsimd.dma_start(out=bc_sb, in_=bc_dr[:])
                bh_sb = p1.tile([128, 6], f32)
                nc.gpsimd.dma_start(out=bh_sb, in_=bh_dr[:])
                bt_sb = p1.tile([128, 6], f32)
                nc.gpsimd.dma_start(out=bt_sb, in_=bt_dr[:])
                S2_sb = p1.tile([128, 30], bft)
                nc.gpsimd.dma_start(out=S2_sb, in_=S2_dr[:])
                nc.gpsimd.dma_start(out=sel4_sb, in_=sel4_dr[:])
                ones_sb = p1.tile([128, 1], bft)
                nc.gpsimd.dma_start(out=ones_sb, in_=ones_dr[:])
                onesrow_sb = p1.tile([1, 128], f32)
                nc.gpsimd.dma_start(out=onesrow_sb, in_=onesrow_dr[:])
                wh1_sb = p1.tile([128, 6, E], bft)
                nc.gpsimd.dma_start(out=wh1_sb, in_=wh1_dr[:].rearrange("(a p) e -> p a e", p=128))
                wt1_sb = p1.tile([128, 6, E], bft)
                nc.gpsimd.dma_start(out=wt1_sb, in_=wt1_dr[:].rearrange("(a p) e -> p a e", p=128))

                # scalar queue: seq + exp/ln first, then the extractor weights
                seq_sb = p1.tile([128, 4, H], bft)
                nc.scalar.dma_start(out=seq_sb, in_=seq_dr[:].rearrange("(a p) h -> p a h", p=128))
                exp_g = p1.tile([128, H], bft)
                nc.scalar.activation(out=exp_g, in_=ment_sb[:], func=AF.Exp)

                # eeT[h, ent] = ln(sum_m exp(ment)) via selector matmul
                eeT = p1.tile([128, 6, 30], bft)
                for ht in range(6):
                    tr = ps1.tile([128, 30], f32, tag="sm1", bufs=2)
                    nc.tensor.matmul(tr, lhsT=exp_g[:, 128 * ht : 128 * (ht + 1)],
                                     rhs=S2_sb[:], start=True, stop=True)
                    nc.scalar.activation(out=eeT[:, ht, :], in_=tr, func=AF.Ln)

                wh2_sb = p1.tile([128, 6, E], bft)
                nc.scalar.dma_start(out=wh2_sb, in_=wh2_dr[:].rearrange("(a p) e -> p a e", p=128))
                wt2_sb = p1.tile([128, 6, E], bft)
                nc.scalar.dma_start(out=wt2_sb, in_=wt2_dr[:].rearrange("(a p) e -> p a e", p=128))

                # ---------- pair attention products (unnormalized) ----------
                # (emitted before the hp/tp copies so the vector queue starts
                # on the critical path as soon as e_att arrives)
                ht_raw = p1.tile([128, 4, PL], bft)
                sum_ps = ps1.tile([1, PL], f32, tag="lsum", bufs=1)
                with nc.allow_low_precision("bf16 pair-product reduce; normalization is scale-invariant"):
                    for lt in range(4):
                        prod = p1.tile([128, IPC, NE, NH], bft, tag="prod", bufs=2)
                        in0 = eaT[:, lt, 24:30, :].unsqueeze(2).broadcast_to([128, IPC, NE, NH])
                        in1 = eaT[:, lt, 0:24, :].unsqueeze(1).broadcast_to([128, IPC, NE, NH])
                        nc.vector.tensor_mul(out=prod, in0=in0, in1=in1)
                        nc.vector.tensor_reduce(
                            out=ht_raw[:, lt, :],
                            in_=prod[:].rearrange("p a b h -> p (a b) h"),
                            axis=mybir.AxisListType.X, op=ALU.add)
                        nc.vector.tensor_scalar_max(
                            out=ht_raw[:, lt, :], in0=ht_raw[:, lt, :], scalar1=0.0)

                # hpartT [E'(perm), own-i 6] (tpartT comes after the hs
                # extractor so its 36 tiny matmuls don't delay the AllToAll)
                hpT = p1.tile([128, 6, IPC], bft)
                tpT = p1.tile([128, 6, NE], bft)
                for Et in range(6):
                    hp = ps1.tile([128, IPC], f32, tag="sm1", bufs=2)
                    for ht in range(6):
                        nc.tensor.matmul(
                            hp, lhsT=wh1_sb[:, ht, 128 * Et : 128 * (Et + 1)],
                            rhs=eeT[:, ht, 24:30], start=(ht == 0), stop=(ht == 5))
                    nc.scalar.copy(out=hpT[:, Et, :], in_=hp)

                for lt in range(4):
                    nc.tensor.matmul(sum_ps, lhsT=ones_sb[:], rhs=ht_raw[:, lt, :],
                                     start=(lt == 0), stop=(lt == 3))
                denom = p1.tile([1, PL], f32)
                nc.vector.tensor_scalar_add(out=denom, in0=sum_ps, scalar1=1e-10)
                recip = p1.tile([1, PL], f32)
                nc.vector.reciprocal(out=recip, in_=denom)
                rep_ps = ps1.tile([128, PL], f32, tag="acc", bufs=2)
                nc.tensor.matmul(rep_ps, lhsT=onesrow_sb[:], rhs=recip[:], start=True, stop=True)
                recip_rep = p1.tile([128, PL], f32)
                nc.vector.tensor_copy(out=recip_rep, in_=rep_ps)

                # ---------- rs^T = seq^T @ ht_raw, scaled by 1/sum on copy-out ----------
                rsT = p1.tile([128, 6, PL], bft)
                for ht in range(6):
                    rp = ps1.tile([128, PL], f32, tag="acc", bufs=2)
                    for lt in range(4):
                        nc.tensor.matmul(rp, lhsT=seq_sb[:, lt, 128 * ht : 128 * (ht + 1)],
                                         rhs=ht_raw[:, lt, :], start=(lt == 0), stop=(lt == 3))
                    nc.vector.tensor_mul(out=rsT[:, ht, :], in0=rp, in1=recip_rep)

                # ---------- extractors, interleaved in E-halves so the CC
                # stream runs A2A1 -> AG1 -> A2A2 -> AG2 and chunk 0 can start
                # right after AG1 ----------
                def hs_half(h0):
                    for Et in range(h0, h0 + 3):
                        hp_b = hpT[:, Et, :].unsqueeze(2).broadcast_to([128, IPC, NE])
                        ep = ps1.tile([128, PL], f32, tag="acc", bufs=2)
                        for ht in range(6):
                            nc.tensor.matmul(ep, lhsT=wh2_sb[:, ht, 128 * Et : 128 * (Et + 1)],
                                             rhs=rsT[:, ht, :], start=(ht == 0), stop=(ht == 5))
                        nc.vector.tensor_add(out=ep[:].rearrange("p (a b) -> p a b", a=IPC),
                                             in0=ep[:].rearrange("p (a b) -> p a b", a=IPC), in1=hp_b)
                        nc.scalar.activation(out=hs_sb[:, Et, :], in_=ep, func=AF.Tanh,
                                             bias=bh_sb[:, Et : Et + 1])
                    sl = slice(h0 * 128, (h0 + 3) * 128)
                    nc.scalar.dma_start(
                        out=hs_cc_in[sl].rearrange("(a p) c -> p a c", p=128),
                        in_=hs_sb[:, h0 : h0 + 3, :])
                    nc.gpsimd.collective_compute(
                        "AllToAll", ALU.bypass, replica_groups=groups,
                        ins=[hs_cc_in[sl].opt()], outs=[hs_cc_out[sl].opt()])

                def ts_half(h0, out_cc):
                    for Et in range(h0, h0 + 3):
                        tp = ps1.tile([128, NE], f32, tag="sm1", bufs=2)
                        for ht in range(6):
                            nc.tensor.matmul(
                                tp, lhsT=wt1_sb[:, ht, 128 * Et : 128 * (Et + 1)],
                                rhs=eeT[:, ht, 0:24], start=(ht == 0), stop=(ht == 5))
                        nc.scalar.copy(out=tpT[:, Et, :], in_=tp)
                    for Et in range(h0, h0 + 3):
                        tp_b = tpT[:, Et, :].unsqueeze(1).broadcast_to([128, IPC, NE])
                        ep2 = ps1.tile([128, PL], f32, tag="acc", bufs=2)
                        for ht in range(6):
                            nc.tensor.matmul(ep2, lhsT=wt2_sb[:, ht, 128 * Et : 128 * (Et + 1)],
                                             rhs=rsT[:, ht, :], start=(ht == 0), stop=(ht == 5))
                        nc.vector.tensor_add(out=ep2[:].rearrange("p (a b) -> p a b", a=IPC),
                                             in0=ep2[:].rearrange("p (a b) -> p a b", a=IPC), in1=tp_b)
                        nc.scalar.activation(out=ts_sb[:, Et, :], in_=ep2, func=AF.Tanh,
                                             bias=bt_sb[:, Et : Et + 1])
                    sl = slice(h0 * 128, (h0 + 3) * 128)
                    nc.scalar.dma_start(
                        out=ts_cc_in[sl].rearrange("(a p) c -> p a c", p=128),
                        in_=ts_sb[:, h0 : h0 + 3, :])
                    nc.gpsimd.collective_compute(
                        "AllGather", ALU.bypass, replica_groups=groups,
                        ins=[ts_cc_in[sl].opt()], outs=[out_cc[:].opt()])

                hs_half(0)
                ts_half(0, ts_cc_o1)
                hs_half(3)
                ts_half(3, ts_cc_o2)

                # redistributed hs resident in SBUF: partition kb*8+s holds, per
                # (u, d): hs value for (k=2u+kb, s) of source-core d's pairs
                for uh in range(2):
                    for d in range(NCORE):
                        for kb in range(2):
                            nc.gpsimd.dma_start(
                                out=hs_all[8 * kb : 8 * (kb + 1), 3 * uh : 3 * uh + 3, d, :],
                                in_=bass.AP(
                                    tensor=hs_cc_out,
                                    offset=(uh * 384 + d * 48 + kb * 24) * PL,
                                    ap=[[3 * PL, 8], [PL, 3], [1, PL]],
                                ),
                            )
                # ts^T of all pairs resident in SBUF: ts_all[p, u, d, :] =
                # ts row 128u+p of source-core d (partition p<64 is (k=2u, t=p),
                # p>=64 is (k=2u+1, t=p-64))
                for u in range(NU):
                    src, uo = (ts_cc_o1, u) if u < 3 else (ts_cc_o2, u - 3)
                    nc.sync.dma_start(
                        out=ts_all[:, u, :, :],
                        in_=bass.AP(
                            tensor=src, offset=uo * 128 * PL,
                            ap=[[PL, 128], [(E // 2) * PL, NCORE], [1, PL]],
                        ),
                    )

            # ---------- phase 2: feature + classifier over pair chunks ----------
            with (
                tc.tile_pool(name="p2", bufs=2) as p2,
                tc.tile_pool(name="ps2", bufs=1, space="PSUM") as ps2,
            ):
                # flat (chunk, tile) schedule: the sel matmul + its scalar
                # PSUM->SBUF bf16 staging copy run a fixed DEPTH iterations
                # ahead of the fps matmuls (crossing chunk boundaries), so no
                # producer-consumer semaphore hop is ever on the critical path
                work = [(c, t) for c in range(NCHUNK) for t in range(NKT)]
                DEPTH = 6
                b1s_ring = {}

                def emit_selcopy(i):
                    c, t = work[i]
                    u, j = t // 8, t % 8
                    ps = ps2.tile([128, CW], f32, tag="b1ps", bufs=2)
                    nc.tensor.matmul(
                        ps, lhsT=sel4_sb[:, j, :],
                        rhs=hs_all[:, u, 2 * c : 2 * c + 2, :].rearrange(
                            "p a b -> p (a b)"),
                        start=True, stop=True)
                    b1s = p2.tile([128, CW], bft, tag="b1s", bufs=DEPTH + 10)
                    nc.scalar.copy(out=b1s, in_=ps)
                    b1s_ring[i] = b1s

                for i in range(DEPTH):
                    emit_selcopy(i)

                fps = []
                for i, (ck, kt) in enumerate(work):
                    if kt == 0:
                        fps = []
                        for h in range(6):
                            fpt = ps2.tile([128, CW], f32, tag=f"feat{h}", bufs=1,
                                           name=f"fps{h}")
                            fps.append(fpt)
                    u = kt // 8
                    bl = p2.tile([128, CW], bft, tag="bl", bufs=4)
                    nc.vector.tensor_mul(
                        out=bl, in0=b1s_ring.pop(i),
                        in1=ts_all[:, u, 2 * ck : 2 * ck + 2, :].rearrange(
                            "p a b -> p (a b)"))
                    for h in range(6):
                        nc.tensor.matmul(
                            fps[h], lhsT=wp_sb[:, kt, 128 * h : 128 * (h + 1)],
                            rhs=bl, start=(kt == 0), stop=(kt == NKT - 1))
                    if i + DEPTH < len(work):
                        emit_selcopy(i + DEPTH)
                    if kt != NKT - 1:
                        continue

                    # classifier
                    lgp = ps2.tile([C, CW], f32, tag="b1ps", bufs=2)
                    for h in range(6):
                        fT = p2.tile([128, CW], bft, tag="fT", bufs=3)
                        if h % 2 == 0:
                            nc.scalar.copy(out=fT, in_=fps[h])
                        else:
                            nc.vector.tensor_copy(out=fT, in_=fps[h])
                        nc.tensor.matmul(lgp, lhsT=wc_sb[:, h, :], rhs=fT,
                                         start=(h == 0), stop=(h == 5))

                    # bias (bc/8 per core) + self-pair mask BEFORE the AllReduce
                    lgs = p2.tile([C, CW], f32, tag="lgs", bufs=2)
                    nc.vector.tensor_scalar_add(out=lgs, in0=lgp, scalar1=bc_sb[:])
                    for col in _self_cols(ck):
                        nc.vector.memset(lgs[:, col : col + 1], 0.0)
                    nc.scalar.dma_start(out=lg_cc_in[ck, :, :], in_=lgs)

                    nc.gpsimd.collective_compute(
                        "AllReduce", ALU.add, replica_groups=groups,
                        ins=[lg_cc_in[ck, :, :].opt()],
                        outs=[lg_cc_out[ck, :, :].opt()])
                    if ck >= 1:
                        nc.scalar.dma_start(
                            out=out_lgT[:, (ck - 1) * CW : ck * CW],
                            in_=lg_cc_out[ck - 1, :, :])
                nc.scalar.dma_start(
                    out=out_lgT[:, (NCHUNK - 1) * CW :],
                    in_=lg_cc_out[NCHUNK - 1, :, :])

    if not nc.is_finalized():
        nc.finalize()
    return nc


_NC_CACHE = None


def kernel(**inputs):
    global _NC_CACHE
    from concourse.bass_utils import run_bass_kernel_spmd

    if _NC_CACHE is None:
        _NC_CACHE = build_bass()
    in_maps = _host_prep(inputs)
    res = run_bass_kernel_spmd(_NC_CACHE, in_maps, core_ids=list(range(NCORE)))
    kernel.last_results = res
    out = res.results[0]["out_lgT"]  # [97, 1152]
    return np.ascontiguousarray(out.T).astype(np.float32)


# revision 14
# speedup vs baseline: 1.4154x; 1.4154x over previous
"""Trainium2 Bass kernel for nn_DocREModel_Triangle (DocRE block-bilinear model).

Strategy (8 NeuronCores, single SPMD NEFF):
  Phase 1 (pair-parallel): core c owns batch b=c//4 and entity rows
  i in [6*(c%4), 6*(c%4)+6) -> 144 (i,j) pairs. Host prep gathers the
  mention rows of sequence_output (for the logsumexp entity pooling)
  and pre-sums the attention rows over the 4 mentions (the /4 mean
  cancels in the normalization), so the device starts from e_att^T
  [l, ent, h] directly. Device computes pairwise head-products, context
  vectors rs^T (the per-pair 1/sum normalization is folded into the
  PSUM->SBUF copy of rs^T, so the rs matmuls don't wait on the
  reciprocal), and the two tanh extractors hs^T/ts^T [768, 144] (bf16),
  all in a transposed layout (feature dim on partitions, pairs free).
  Collectives (single stream, pipelined against compute): hs extractor
  runs first and an AllToAll redistributes hs^T by s-slices; the ts
  extractor follows with its AllGather split in two halves so AG1 can
  start while Et 3-5 still compute.
  Phase 2 (contraction-parallel): core c holds Wp rows (k, s in
  [8c,8c+8), t) resident in SBUF (bf16, 9.4 MB), with rows retiled so
  each 128-row tile pairs (k=2u, s=j) on partitions 0-63 with
  (k=2u+1, s=j) on partitions 64-127 -- that makes the ts operand of
  the bilinear product a direct slice of the AllGather result in SBUF
  (no replication DMA). hs rows are emitted in (j-dest, kb, s, u) order
  (host-side column permutation of Wh) so the whole AllToAll result
  loads into SBUF with 16 clean DMAs. A selector matmul broadcasts the
  two hs values over the 64 t-positions, emitted two tiles ahead of the
  6 Wp-contraction matmuls so the tensor queue never stalls on the DVE
  multiply; 4 pair-chunks of 288, then the classifier. Bias (bc/8) and
  the self-pair mask are applied before the per-chunk logits AllReduce,
  so the post-AllReduce step is a pure DRAM->DRAM copy to the output.
"""

import numpy as np
import ml_dtypes

bf16 = ml_dtypes.bfloat16

B, L, H, NH = 2, 512, 768, 12
NE, NM = 24, 4
E, BS, C = 768, 64, 97
K = E // BS                      # 12 blocks
NCORE = 8
IPC = NE // 4                    # 6 i-rows per core (4 cores per batch elem)
PL = IPC * NE                    # 144 local pairs
NP = B * NE * NE                 # 1152 global pairs
SL = 64 // NCORE                 # 8 s-values per core
KST = K * SL * BS                # 6144 contraction rows per core
NKT = KST // 128                 # 48 contraction tiles
NU = K // 2                      # 6 k-pairs (u)
NCHUNK = 4                       # pair chunks of 288 (= 2 pair-blocks)
CW = NP // NCHUNK                # 288

# hs row order: e' = uh*384 + j*48 + kb*24 + s_l*3 + u'  (u = 3*uh + u')
# <- e = (2u+kb)*64 + 8j + s_l.  uh outermost lets the AllToAll run as two
# halves pipelined with the hs extractor; j = destination core of each A2A
# slice; the (kb, s, u') nesting gives phase 2 single-stride loads
_PERM2 = np.zeros(E, dtype=np.int64)
for _uh in range(2):
    for _j in range(8):
        for _kb in range(2):
            for _s in range(8):
                for _up in range(3):
                    _PERM2[_uh * 384 + _j * 48 + _kb * 24 + _s * 3 + _up] = (
                        2 * (3 * _uh + _up) + _kb) * 64 + 8 * _j + _s


def _host_prep(inputs):
    """Build the 8 per-core input maps from the full inputs."""
    seq = np.ascontiguousarray(inputs["sequence_output"], dtype=np.float32)
    att = np.ascontiguousarray(inputs["attention"], dtype=np.float32)
    Wh = np.asarray(inputs["Wh"], dtype=np.float32)
    bh = np.asarray(inputs["bh"], dtype=np.float32)
    Wt = np.asarray(inputs["Wt"], dtype=np.float32)
    bt = np.asarray(inputs["bt"], dtype=np.float32)
    Wp = np.asarray(inputs["Wp"], dtype=np.float32)
    Wc = np.asarray(inputs["Wc"], dtype=np.float32)
    bc = np.asarray(inputs["bc"], dtype=np.float32)
    mpos = np.asarray(inputs["mention_pos"]).astype(np.int64)

    wh1p = np.ascontiguousarray(Wh[:H][:, _PERM2].astype(bf16))
    wh2p = np.ascontiguousarray(Wh[H:][:, _PERM2].astype(bf16))
    wt1 = np.ascontiguousarray(Wt[:H].astype(bf16))
    wt2 = np.ascontiguousarray(Wt[H:].astype(bf16))
    bh_p = np.ascontiguousarray(bh[_PERM2].reshape(6, 128).T.astype(np.float32))
    bt_t = np.ascontiguousarray(bt.reshape(6, 128).T.astype(np.float32))
    wc_bf = np.ascontiguousarray(Wc.astype(bf16))
    bc8 = np.ascontiguousarray((bc / NCORE).reshape(C, 1).astype(np.float32))
    wp4 = Wp.reshape(K, 64, BS, H)

    in_maps = []
    for c in range(NCORE):
        b = c // 4
        i0 = (c % 4) * IPC
        ents = list(range(NE)) + list(range(i0, i0 + IPC))  # 24 j-side + 6 i-side

        # host-gathered mention rows of seq: 4 m-blocks at 32-partition alignment
        rows = seq[b][mpos[b, ents, :]]                   # [30, NM, H]
        ment = np.zeros((128, H), np.float32)
        ment.reshape(NM, 32, H)[:, :30] = rows.transpose(1, 0, 2)

        # host mention-sum of attention rows -> e_att^T [L, 30, NH]
        A = att[b][:, mpos[b, ents, :], :]                # [NH, 30, NM, L]
        e_attT = np.ascontiguousarray(A.sum(2).transpose(2, 1, 0).astype(bf16))

        # Wp rows for this core's s-slice, retiled so 128-row tile (u*8+j)
        # is [(k=2u, s=8c+j, t 0..63), (k=2u+1, s=8c+j, t 0..63)]
        wpc = wp4[:, SL * c : SL * (c + 1)]               # [K, 8, 64, H]
        wp_sl = np.ascontiguousarray(
            wpc.reshape(NU, 2, SL, BS, H).transpose(0, 2, 1, 3, 4)
            .reshape(KST, H).astype(bf16)
        )

        in_maps.append(
            {
                "ment": np.ascontiguousarray(ment),
                "e_attT": e_attT,
                "seq_bf": np.ascontiguousarray(seq[b].astype(bf16)),
                "wh1p": wh1p,
                "wh2p": wh2p,
                "wt1": wt1,
                "wt2": wt2,
                "bh_p": bh_p,
                "bt_t": bt_t,
                "wp_sl": wp_sl,
                "wc_bf": wc_bf,
                "bc8": bc8,
            }
        )
    return in_maps


def _build_consts():
    # S2 [128, 30]: sums the 4 mention-row exps per entity during the
    # logsumexp transpose-matmul
    S2 = np.zeros((128, 30), dtype=bf16)
    for m in range(NM):
        for e_i in range(30):
            S2[m * 32 + e_i, e_i] = 1.0
    ones_bf = np.ones((128, 1), dtype=bf16)
    ones_row = np.ones((1, 128), dtype=np.float32)
    # sel4 [16, 8, 128]: broadcasts hs_all partition j (k even) to
    # partitions 0-63 and partition 8+j (k odd) to partitions 64-127
    sel4 = np.zeros((16, 8, 128), dtype=bf16)
    for j in range(8):
        sel4[j, j, 0:64] = 1
        sel4[8 + j, j, 64:128] = 1
    return S2, ones_bf, ones_row, sel4


def _self_cols(ck):
    """Local column indices of self-pairs (i==j) within chunk ck."""
    cols = []
    for d in range(2):
        c = 2 * ck + d
        for il in range(IPC):
            ig = (c % 4) * IPC + il
            cols.append(d * PL + il * NE + ig)
    return cols


def build_bass():
    import concourse.bass as bass
    import concourse.mybir as mybir
    import concourse.tile as tile
    from concourse.bacc import Bacc

    f32 = mybir.dt.float32
    bft = mybir.dt.bfloat16
    AF = mybir.ActivationFunctionType
    ALU = mybir.AluOpType

    nc = Bacc("TRN2", num_devices=NCORE)

    # ---- I/O ----
    ment_dr = nc.dram_tensor("ment", [128, H], f32, kind="ExternalInput")
    eat_dr = nc.dram_tensor("e_attT", [L, 30, NH], bft, kind="ExternalInput")
    seq_dr = nc.dram_tensor("seq_bf", [L, H], bft, kind="ExternalInput")
    wh1_dr = nc.dram_tensor("wh1p", [H, E], bft, kind="ExternalInput")
    wh2_dr = nc.dram_tensor("wh2p", [H, E], bft, kind="ExternalInput")
    wt1_dr = nc.dram_tensor("wt1", [H, E], bft, kind="ExternalInput")
    wt2_dr = nc.dram_tensor("wt2", [H, E], bft, kind="ExternalInput")
    bh_dr = nc.dram_tensor("bh_p", [128, 6], f32, kind="ExternalInput")
    bt_dr = nc.dram_tensor("bt_t", [128, 6], f32, kind="ExternalInput")
    wp_dr = nc.dram_tensor("wp_sl", [KST, H], bft, kind="ExternalInput")
    wc_dr = nc.dram_tensor("wc_bf", [H, C], bft, kind="ExternalInput")
    bc_dr = nc.dram_tensor("bc8", [C, 1], f32, kind="ExternalInput")
    out_lgT = nc.dram_tensor("out_lgT", [C, NP], f32, kind="ExternalOutput")

    S2_np, ones_np, onesrow_np, sel4_np = _build_consts()
    S2_dr = nc.inline_tensor(S2_np, "s2_const")
    ones_dr = nc.inline_tensor(ones_np, "ones_const")
    onesrow_dr = nc.inline_tensor(onesrow_np, "onesrow_const")
    sel4_dr = nc.inline_tensor(sel4_np, "sel4_const")

    # collective buffers
    hs_cc_in = nc.dram_tensor("hs_cc_in", [E, PL], bft)
    hs_cc_out = nc.dram_tensor("hs_cc_out", [E, PL], bft)
    ts_cc_in = nc.dram_tensor("ts_cc_in", [E, PL], bft)
    ts_cc_o1 = nc.dram_tensor("ts_cc_o1", [NCORE, E // 2, PL], bft, addr_space="Shared")
    ts_cc_o2 = nc.dram_tensor("ts_cc_o2", [NCORE, E // 2, PL], bft, addr_space="Shared")
    lg_cc_in = nc.dram_tensor("lg_cc_in", [NCHUNK, C, CW], f32)
    lg_cc_out = nc.dram_tensor("lg_cc_out", [NCHUNK, C, CW], f32, addr_space="Shared")
    groups = [list(range(NCORE))]

    with tile.TileContext(nc) as tc:
        with (
            tc.tile_pool(name="gpool", bufs=1) as gpool,
            tc.tile_pool(name="persist", bufs=1) as persist,
        ):
            # ---------- whole-kernel-lifetime weights / constants ----------
            wp_sb = gpool.tile([128, NKT, H], bft)
            wc_sb = gpool.tile([128, 6, C], bft)
            bc_sb = gpool.tile([C, 1], f32)
            sel4_sb = gpool.tile([16, 8, 128], bft)
            ts_all = gpool.tile([128, NU, NCORE, PL], bft)
            hs_all = gpool.tile([16, NU, NCORE, PL], bft)

            hs_sb = persist.tile([128, 6, PL], bft)
            ts_sb = persist.tile([128, 6, PL], bft)

            with (
                tc.tile_pool(name="p1", bufs=1) as p1,
                tc.tile_pool(name="ps1", bufs=2, space="PSUM") as ps1,
            ):
                # ---------- DMA schedule ----------
                # sync queue: ment + e_att (critical), then the big Wp load
                ment_sb = p1.tile([128, H], f32)
                nc.sync.dma_start(out=ment_sb, in_=ment_dr[:])
                eaT = p1.tile([128, 4, 30, NH], bft)
                for lt in range(4):
                    nc.sync.dma_start(out=eaT[:, lt, :, :],
                                      in_=eat_dr[lt * 128 : (lt + 1) * 128])
                for q in range(4):
                    r0 = q * (KST // 4)
                    nc.sync.dma_start(
                        out=wp_sb[:, q * (NKT // 4) : (q + 1) * (NKT // 4), :],
                        in_=wp_dr[r0 : r0 + KST // 4].rearrange("(a p) h -> p a h", p=128),
                    )
                nc.sync.dma_start(out=wc_sb, in_=wc_dr[:].rearrange("(a p) c -> p a c", p=128))

                # gpsimd queue: small consts then phase-1 h-side weights
                nc.gpsimd.dma_start(out=bc_sb, in_=bc_dr[:])
                bh_sb = p1.tile([128, 6], f32)
                nc.gpsimd.dma_start(out=bh_sb, in_=bh_dr[:])
                bt_sb = p1.tile([128, 6], f32)
                nc.gpsimd.dma_start(out=bt_sb, in_=bt_dr[:])
                S2_sb = p1.tile([128, 30], bft)
                nc.gpsimd.dma_start(out=S2_sb, in_=S2_dr[:])
                nc.gpsimd.dma_start(out=sel4_sb, in_=sel4_dr[:])
                ones_sb = p1.tile([128, 1], bft)
                nc.gpsimd.dma_start(out=ones_sb, in_=ones_dr[:])
                onesrow_sb = p1.tile([1, 128], f32)
                nc.gpsimd.dma_start(out=onesrow_sb, in_=onesrow_dr[:])
                wh1_sb = p1.tile([128, 6, E], bft)
                nc.gpsimd.dma_start(out=wh1_sb, in_=wh1_dr[:].rearrange("(a p) e -> p a e", p=128))
                wt1_sb = p1.tile([128, 6, E], bft)
                nc.gpsimd.dma_start(out=wt1_sb, in_=wt1_dr[:].rearrange("(a p) e -> p a e", p=128))

                # scalar queue: seq + exp/ln first, then the extractor weights
                seq_sb = p1.tile([128, 4, H], bft)
                nc.scalar.dma_start(out=seq_sb, in_=seq_dr[:].rearrange("(a p) h -> p a h", p=128))
                exp_g = p1.tile([128, H], bft)
                nc.scalar.activation(out=exp_g, in_=ment_sb[:], func=AF.Exp)

                # eeT[h, ent] = ln(sum_m exp(ment)) via selector matmul
                eeT = p1.tile([128, 6, 30], bft)
                for ht in range(6):
                    tr = ps1.tile([128, 30], f32, tag="sm1", bufs=2)
                    nc.tensor.matmul(tr, lhsT=exp_g[:, 128 * ht : 128 * (ht + 1)],
                                     rhs=S2_sb[:], start=True, stop=True)
                    nc.scalar.activation(out=eeT[:, ht, :], in_=tr, func=AF.Ln)

                wh2_sb = p1.tile([128, 6, E], bft)
                nc.scalar.dma_start(out=wh2_sb, in_=wh2_dr[:].rearrange("(a p) e -> p a e", p=128))
                wt2_sb = p1.tile([128, 6, E], bft)
                nc.scalar.dma_start(out=wt2_sb, in_=wt2_dr[:].rearrange("(a p) e -> p a e", p=128))

                # ---------- pair attention products (unnormalized) ----------
                # (emitted before the hp/tp copies so the vector queue starts
                # on the critical path as soon as e_att arrives)
                ht_raw = p1.tile([128, 4, PL], bft)
                sum_ps = ps1.tile([1, PL], f32, tag="lsum", bufs=1)
                with nc.allow_low_precision("bf16 pair-product reduce; normalization is scale-invariant"):
                    for lt in range(4):
                        prod = p1.tile([128, IPC, NE, NH], bft, tag="prod", bufs=2)
                        in0 = eaT[:, lt, 24:30, :].unsqueeze(2).broadcast_to([128, IPC, NE, NH])
                        in1 = eaT[:, lt, 0:24, :].unsqueeze(1).broadcast_to([128, IPC, NE, NH])
                        nc.vector.tensor_mul(out=prod, in0=in0, in1=in1)
                        nc.vector.tensor_reduce(
                            out=ht_raw[:, lt, :],
                            in_=prod[:].rearrange("p a b h -> p (a b) h"),
                            axis=mybir.AxisListType.X, op=ALU.add)
                        nc.vector.tensor_scalar_max(
                            out=ht_raw[:, lt, :], in0=ht_raw[:, lt, :], scalar1=0.0)

                # hpartT [E'(perm), own-i 6] (tpartT comes after the hs
                # extractor so its 36 tiny matmuls don't delay the AllToAll)
                hpT = p1.tile([128, 6, IPC], bft)
                tpT = p1.tile([128, 6, NE], bft)
                for Et in range(6):
                    hp = ps1.tile([128, IPC], f32, tag="sm1", bufs=2)
                    for ht in range(6):
                        nc.tensor.matmul(
                            hp, lhsT=wh1_sb[:, ht, 128 * Et : 128 * (Et + 1)],
                            rhs=eeT[:, ht, 24:30], start=(ht == 0), stop=(ht == 5))
                    nc.scalar.copy(out=hpT[:, Et, :], in_=hp)

                for lt in range(4):
                    nc.tensor.matmul(sum_ps, lhsT=ones_sb[:], rhs=ht_raw[:, lt, :],
                                     start=(lt == 0), stop=(lt == 3))
                denom = p1.tile([1, PL], f32)
                nc.vector.tensor_scalar_add(out=denom, in0=sum_ps, scalar1=1e-10)
                recip = p1.tile([1, PL], f32)
                nc.vector.reciprocal(out=recip, in_=denom)
                rep_ps = ps1.tile([128, PL], f32, tag="acc", bufs=2)
                nc.tensor.matmul(rep_ps, lhsT=onesrow_sb[:], rhs=recip[:], start=True, stop=True)
                recip_rep = p1.tile([128, PL], f32)
                nc.vector.tensor_copy(out=recip_rep, in_=rep_ps)

                # ---------- rs^T = seq^T @ ht_raw, scaled by 1/sum on copy-out ----------
                rsT = p1.tile([128, 6, PL], bft)
                for ht in range(6):
                    rp = ps1.tile([128, PL], f32, tag="acc", bufs=2)
                    for lt in range(4):
                        nc.tensor.matmul(rp, lhsT=seq_sb[:, lt, 128 * ht : 128 * (ht + 1)],
                                         rhs=ht_raw[:, lt, :], start=(lt == 0), stop=(lt == 3))
                    nc.vector.tensor_mul(out=rsT[:, ht, :], in0=rp, in1=recip_rep)

                # ---------- extractors, interleaved in E-halves so the CC
                # stream runs A2A1 -> AG1 -> A2A2 -> AG2 and chunk 0 can start
                # right after AG1 ----------
                def hs_half(h0):
                    for Et in range(h0, h0 + 3):
                        hp_b = hpT[:, Et, :].unsqueeze(2).broadcast_to([128, IPC, NE])
                        ep = ps1.tile([128, PL], f32, tag="acc", bufs=2)
                        for ht in range(6):
                            nc.tensor.matmul(ep, lhsT=wh2_sb[:, ht, 128 * Et : 128 * (Et + 1)],
                                             rhs=rsT[:, ht, :], start=(ht == 0), stop=(ht == 5))
                        nc.vector.tensor_add(out=ep[:].rearrange("p (a b) -> p a b", a=IPC),
                                             in0=ep[:].rearrange("p (a b) -> p a b", a=IPC), in1=hp_b)
                        nc.scalar.activation(out=hs_sb[:, Et, :], in_=ep, func=AF.Tanh,
                                             bias=bh_sb[:, Et : Et + 1])
                    sl = slice(h0 * 128, (h0 + 3) * 128)
                    nc.scalar.dma_start(
                        out=hs_cc_in[sl].rearrange("(a p) c -> p a c", p=128),
                        in_=hs_sb[:, h0 : h0 + 3, :])
                    nc.gpsimd.collective_compute(
                        "AllToAll", ALU.bypass, replica_groups=groups,
                        ins=[hs_cc_in[sl].opt()], outs=[hs_cc_out[sl].opt()])

                def ts_half(h0, out_cc):
                    for Et in range(h0, h0 + 3):
                        tp = ps1.tile([128, NE], f32, tag="sm1", bufs=2)
                        for ht in range(6):
                            nc.tensor.matmul(
                                tp, lhsT=wt1_sb[:, ht, 128 * Et : 128 * (Et + 1)],
                                rhs=eeT[:, ht, 0:24], start=(ht == 0), stop=(ht == 5))
                        nc.scalar.copy(out=tpT[:, Et, :], in_=tp)
                    for Et in range(h0, h0 + 3):
                        tp_b = tpT[:, Et, :].unsqueeze(1).broadcast_to([128, IPC, NE])
                        ep2 = ps1.tile([128, PL], f32, tag="acc", bufs=2)
                        for ht in range(6):
                            nc.tensor.matmul(ep2, lhsT=wt2_sb[:, ht, 128 * Et : 128 * (Et + 1)],
                                             rhs=rsT[:, ht, :], start=(ht == 0), stop=(ht == 5))
                        nc.vector.tensor_add(out=ep2[:].rearrange("p (a b) -> p a b", a=IPC),
                                             in0=ep2[:].rearrange("p (a b) -> p a b", a=IPC), in1=tp_b)
                        nc.scalar.activation(out=ts_sb[:, Et, :], in_=ep2, func=AF.Tanh,
                                             bias=bt_sb[:, Et : Et + 1])
                    sl = slice(h0 * 128, (h0 + 3) * 128)
                    nc.scalar.dma_start(
                        out=ts_cc_in[sl].rearrange("(a p) c -> p a c", p=128),
                        in_=ts_sb[:, h0 : h0 + 3, :])
                    nc.gpsimd.collective_compute(
                        "AllGather", ALU.bypass, replica_groups=groups,
                        ins=[ts_cc_in[sl].opt()], outs=[out_cc[:].opt()])

                hs_half(0)
                ts_half(0, ts_cc_o1)
                hs_half(3)
                ts_half(3, ts_cc_o2)

                # redistributed hs resident in SBUF: partition kb*8+s holds, per
                # (u, d): hs value for (k=2u+kb, s) of source-core d's pairs
                for uh in range(2):
                    for d in range(NCORE):
                        for kb in range(2):
                            nc.gpsimd.dma_start(
                                out=hs_all[8 * kb : 8 * (kb + 1), 3 * uh : 3 * uh + 3, d, :],
                                in_=bass.AP(
                                    tensor=hs_cc_out,
                                    offset=(uh * 384 + d * 48 + kb * 24) * PL,
                                    ap=[[3 * PL, 8], [PL, 3], [1, PL]],
                                ),
                            )
                # ts^T of all pairs resident in SBUF: ts_all[p, u, d, :] =
                # ts row 128u+p of source-core d (partition p<64 is (k=2u, t=p),
                # p>=64 is (k=2u+1, t=p-64))
                for u in range(NU):
                    src, uo = (ts_cc_o1, u) if u < 3 else (ts_cc_o2, u - 3)
                    nc.sync.dma_start(
                        out=ts_all[:, u, :, :],
                        in_=bass.AP(
                            tensor=src, offset=uo * 128 * PL,
                            ap=[[PL, 128], [(E // 2) * PL, NCORE], [1, PL]],
                        ),
                    )

            # ---------- phase 2: feature + classifier over pair chunks ----------
            with (
                tc.tile_pool(name="p2", bufs=2) as p2,
                tc.tile_pool(name="ps2", bufs=1, space="PSUM") as ps2,
            ):
                # flat (chunk, tile) schedule: the sel matmul + its scalar
                # PSUM->SBUF bf16 staging copy run a fixed DEPTH iterations
                # ahead of the fps matmuls (crossing chunk boundaries), so no
                # producer-consumer semaphore hop is ever on the critical path
                work = [(c, t) for c in range(NCHUNK) for t in range(NKT)]
                DEPTH = 6
                b1s_ring = {}

                def emit_selcopy(i):
                    c, t = work[i]
                    u, j = t // 8, t % 8
                    ps = ps2.tile([128, CW], f32, tag="b1ps", bufs=2)
                    nc.tensor.matmul(
                        ps, lhsT=sel4_sb[:, j, :],
                        rhs=hs_all[:, u, 2 * c : 2 * c + 2, :].rearrange(
                            "p a b -> p (a b)"),
                        start=True, stop=True)
                    b1s = p2.tile([128, CW], bft, tag="b1s", bufs=DEPTH + 10)
                    nc.scalar.copy(out=b1s, in_=ps)
                    b1s_ring[i] = b1s

                for i in range(DEPTH):
                    emit_selcopy(i)

                fps = []
                for i, (ck, kt) in enumerate(work):
                    if kt == 0:
                        fps = []
                        for h in range(6):
                            fpt = ps2.tile([128, CW], f32, tag=f"feat{h}", bufs=1,
                                           name=f"fps{h}")
                            fps.append(fpt)
                    u = kt // 8
                    bl = p2.tile([128, CW], bft, tag="bl", bufs=4)
                    nc.vector.tensor_mul(
                        out=bl, in0=b1s_ring.pop(i),
                        in1=ts_all[:, u, 2 * ck : 2 * ck + 2, :].rearrange(
                            "p a b -> p (a b)"))
                    for h in range(6):
                        nc.tensor.matmul(
                            fps[h], lhsT=wp_sb[:, kt, 128 * h : 128 * (h + 1)],
                            rhs=bl, start=(kt == 0), stop=(kt == NKT - 1))
                    if i + DEPTH < len(work):
                        emit_selcopy(i + DEPTH)
                    if kt != NKT - 1:
                        continue

                    # classifier
                    lgp = ps2.tile([C, CW], f32, tag="b1ps", bufs=2)
                    for h in range(6):
                        fT = p2.tile([128, CW], bft, tag="fT", bufs=3)
                        if h % 2 == 0:
                            nc.scalar.copy(out=fT, in_=fps[h])
                        else:
                            nc.vector.tensor_copy(out=fT, in_=fps[h])
                        nc.tensor.matmul(lgp, lhsT=wc_sb[:, h, :], rhs=fT,
                                         start=(h == 0), stop=(h == 5))

                    # bias (bc/8 per core) + self-pair mask BEFORE the AllReduce
                    lgs = p2.tile([C, CW], f32, tag="lgs", bufs=2)
                    nc.vector.tensor_scalar_add(out=lgs, in0=lgp, scalar1=bc_sb[:])
                    for col in _self_cols(ck):
                        nc.vector.memset(lgs[:, col : col + 1], 0.0)
                    nc.scalar.dma_start(out=lg_cc_in[ck, :, :], in_=lgs)

                    nc.gpsimd.collective_compute(
                        "AllReduce", ALU.add, replica_groups=groups,
                        ins=[lg_cc_in[ck, :, :].opt()],
                        outs=[lg_cc_out[ck, :, :].opt()])
                    if ck >= 1:
                        nc.scalar.dma_start(
                            out=out_lgT[:, (ck - 1) * CW : ck * CW],
                            in_=lg_cc_out[ck - 1, :, :])
                nc.scalar.dma_start(
                    out=out_lgT[:, (NCHUNK - 1) * CW :],
                    in_=lg_cc_out[NCHUNK - 1, :, :])

    if not nc.is_finalized():
        nc.finalize()
    return nc


_NC_CACHE = None


def kernel(**inputs):
    global _NC_CACHE
    from concourse.bass_utils import run_bass_kernel_spmd

    if _NC_CACHE is None:
        _NC_CACHE = build_bass()
    in_maps = _host_prep(inputs)
    res = run_bass_kernel_spmd(_NC_CACHE, in_maps, core_ids=list(range(NCORE)))
    kernel.last_results = res
    out = res.results[0]["out_lgT"]  # [97, 1152]
    return np.ascontiguousarray(out.T).astype(np.float32)


# revision 15
# speedup vs baseline: 1.5457x; 1.0920x over previous
"""Trainium2 Bass kernel for nn_DocREModel_Triangle (DocRE block-bilinear model).

Strategy (8 NeuronCores, single SPMD NEFF):
  Phase 1 (pair-parallel): core c owns batch b=c//4 and entity rows
  i in [6*(c%4), 6*(c%4)+6) -> 144 (i,j) pairs. Host prep gathers the
  mention rows of sequence_output (for the logsumexp entity pooling)
  and pre-sums the attention rows over the 4 mentions (the /4 mean
  cancels in the normalization), so the device starts from e_att^T
  [l, ent, h] directly. Device computes pairwise head-products, context
  vectors rs^T (the per-pair 1/sum normalization is folded into the
  PSUM->SBUF copy of rs^T, so the rs matmuls don't wait on the
  reciprocal), and the two tanh extractors hs^T/ts^T [768, 144] (bf16),
  all in a transposed layout (feature dim on partitions, pairs free).
  Collectives (single stream, pipelined against compute): hs extractor
  runs first and an AllToAll redistributes hs^T by s-slices; the ts
  extractor follows with its AllGather split in two halves so AG1 can
  start while Et 3-5 still compute.
  Phase 2 (contraction-parallel): core c holds Wp rows (k, s in
  [8c,8c+8), t) resident in SBUF (bf16, 9.4 MB), with rows retiled so
  each 128-row tile pairs (k=2u, s=j) on partitions 0-63 with
  (k=2u+1, s=j) on partitions 64-127 -- that makes the ts operand of
  the bilinear product a direct slice of the AllGather result in SBUF
  (no replication DMA). hs rows are emitted in (j-dest, kb, s, u) order
  (host-side column permutation of Wh) so the whole AllToAll result
  loads into SBUF with 16 clean DMAs. A selector matmul broadcasts the
  two hs values over the 64 t-positions, emitted two tiles ahead of the
  6 Wp-contraction matmuls so the tensor queue never stalls on the DVE
  multiply; 4 pair-chunks of 288, then the classifier. Bias (bc/8) and
  the self-pair mask are applied before the per-chunk logits AllReduce,
  so the post-AllReduce step is a pure DRAM->DRAM copy to the output.
"""

import numpy as np
import ml_dtypes

bf16 = ml_dtypes.bfloat16

B, L, H, NH = 2, 512, 768, 12
NE, NM = 24, 4
E, BS, C = 768, 64, 97
K = E // BS                      # 12 blocks
NCORE = 8
IPC = NE // 4                    # 6 i-rows per core (4 cores per batch elem)
PL = IPC * NE                    # 144 local pairs
NP = B * NE * NE                 # 1152 global pairs
SL = 64 // NCORE                 # 8 s-values per core
KST = K * SL * BS                # 6144 contraction rows per core
NKT = KST // 128                 # 48 contraction tiles
NU = K // 2                      # 6 k-pairs (u)
NCHUNK = 4                       # pair chunks of 288 (= 2 pair-blocks)
CW = NP // NCHUNK                # 288

# hs row order: e' = uh*384 + j*48 + kb*24 + s_l*3 + u'  (u = 3*uh + u')
# <- e = (2u+kb)*64 + 8j + s_l.  uh outermost lets the AllToAll run as two
# halves pipelined with the hs extractor; j = destination core of each A2A
# slice; the (kb, s, u') nesting gives phase 2 single-stride loads
_PERM2 = np.zeros(E, dtype=np.int64)
for _uh in range(2):
    for _j in range(8):
        for _kb in range(2):
            for _s in range(8):
                for _up in range(3):
                    _PERM2[_uh * 384 + _j * 48 + _kb * 24 + _s * 3 + _up] = (
                        2 * (3 * _uh + _up) + _kb) * 64 + 8 * _j + _s


def _host_prep(inputs):
    """Build the 8 per-core input maps from the full inputs."""
    seq = np.ascontiguousarray(inputs["sequence_output"], dtype=np.float32)
    att = np.ascontiguousarray(inputs["attention"], dtype=np.float32)
    Wh = np.asarray(inputs["Wh"], dtype=np.float32)
    bh = np.asarray(inputs["bh"], dtype=np.float32)
    Wt = np.asarray(inputs["Wt"], dtype=np.float32)
    bt = np.asarray(inputs["bt"], dtype=np.float32)
    Wp = np.asarray(inputs["Wp"], dtype=np.float32)
    Wc = np.asarray(inputs["Wc"], dtype=np.float32)
    bc = np.asarray(inputs["bc"], dtype=np.float32)
    mpos = np.asarray(inputs["mention_pos"]).astype(np.int64)

    wh1p = np.ascontiguousarray(Wh[:H][:, _PERM2].astype(bf16))
    wh2p = np.ascontiguousarray(Wh[H:][:, _PERM2].astype(bf16))
    wt1 = np.ascontiguousarray(Wt[:H].astype(bf16))
    wt2 = np.ascontiguousarray(Wt[H:].astype(bf16))
    bh_p = np.ascontiguousarray(bh[_PERM2].reshape(6, 128).T.astype(np.float32))
    bt_t = np.ascontiguousarray(bt.reshape(6, 128).T.astype(np.float32))
    wc_bf = np.ascontiguousarray(Wc.astype(bf16))
    bc8 = np.ascontiguousarray((bc / NCORE).reshape(C, 1).astype(np.float32))
    wp4 = Wp.reshape(K, 64, BS, H)

    in_maps = []
    for c in range(NCORE):
        b = c // 4
        i0 = (c % 4) * IPC
        ents = list(range(NE)) + list(range(i0, i0 + IPC))  # 24 j-side + 6 i-side

        # host-gathered mention rows of seq: 4 m-blocks at 32-partition alignment
        rows = seq[b][mpos[b, ents, :]]                   # [30, NM, H]
        ment = np.zeros((128, H), np.float32)
        ment.reshape(NM, 32, H)[:, :30] = rows.transpose(1, 0, 2)

        # host mention-sum of attention rows -> e_att^T [L, 30, NH]
        A = att[b][:, mpos[b, ents, :], :]                # [NH, 30, NM, L]
        e_attT = np.ascontiguousarray(A.sum(2).transpose(2, 1, 0).astype(bf16))

        # Wp rows for this core's s-slice, retiled so 128-row tile (u*8+j)
        # is [(k=2u, s=8c+j, t 0..63), (k=2u+1, s=8c+j, t 0..63)]
        wpc = wp4[:, SL * c : SL * (c + 1)]               # [K, 8, 64, H]
        wp_sl = np.ascontiguousarray(
            wpc.reshape(NU, 2, SL, BS, H).transpose(0, 2, 1, 3, 4)
            .reshape(KST, H).astype(bf16)
        )

        in_maps.append(
            {
                "ment": np.ascontiguousarray(ment),
                "e_attT": e_attT,
                "seq_bf": np.ascontiguousarray(seq[b].astype(bf16)),
                "wh1p": wh1p,
                "wh2p": wh2p,
                "wt1": wt1,
                "wt2": wt2,
                "bh_p": bh_p,
                "bt_t": bt_t,
                "wp_sl": wp_sl,
                "wc_bf": wc_bf,
                "bc8": bc8,
            }
        )
    return in_maps


def _build_consts():
    # S2 [128, 30]: sums the 4 mention-row exps per entity during the
    # logsumexp transpose-matmul
    S2 = np.zeros((128, 30), dtype=bf16)
    for m in range(NM):
        for e_i in range(30):
            S2[m * 32 + e_i, e_i] = 1.0
    ones_bf = np.ones((128, 1), dtype=bf16)
    ones_row = np.ones((1, 128), dtype=np.float32)
    # sel4 [128, 8, 128]: broadcasts hs_all partition j (k even) to
    # partitions 0-63 and partition 8+j (k odd) to partitions 64-127; rows
    # 16-127 are zero so the sel matmul contracts over all 128 PE rows (a
    # 16-row q0 matmul between full matmuls costs LDWEIGHTS row-group
    # drains on both edges)
    sel4 = np.zeros((128, 8, 128), dtype=bf16)
    for j in range(8):
        sel4[j, j, 0:64] = 1
        sel4[8 + j, j, 64:128] = 1
    return S2, ones_bf, ones_row, sel4


def _self_cols(ck):
    """Local column indices of self-pairs (i==j) within chunk ck."""
    cols = []
    for d in range(2):
        c = 2 * ck + d
        for il in range(IPC):
            ig = (c % 4) * IPC + il
            cols.append(d * PL + il * NE + ig)
    return cols


def build_bass():
    import concourse.bass as bass
    import concourse.mybir as mybir
    import concourse.tile as tile
    from concourse.bacc import Bacc

    f32 = mybir.dt.float32
    bft = mybir.dt.bfloat16
    AF = mybir.ActivationFunctionType
    ALU = mybir.AluOpType

    nc = Bacc("TRN2", num_devices=NCORE)

    # ---- I/O ----
    ment_dr = nc.dram_tensor("ment", [128, H], f32, kind="ExternalInput")
    eat_dr = nc.dram_tensor("e_attT", [L, 30, NH], bft, kind="ExternalInput")
    seq_dr = nc.dram_tensor("seq_bf", [L, H], bft, kind="ExternalInput")
    wh1_dr = nc.dram_tensor("wh1p", [H, E], bft, kind="ExternalInput")
    wh2_dr = nc.dram_tensor("wh2p", [H, E], bft, kind="ExternalInput")
    wt1_dr = nc.dram_tensor("wt1", [H, E], bft, kind="ExternalInput")
    wt2_dr = nc.dram_tensor("wt2", [H, E], bft, kind="ExternalInput")
    bh_dr = nc.dram_tensor("bh_p", [128, 6], f32, kind="ExternalInput")
    bt_dr = nc.dram_tensor("bt_t", [128, 6], f32, kind="ExternalInput")
    wp_dr = nc.dram_tensor("wp_sl", [KST, H], bft, kind="ExternalInput")
    wc_dr = nc.dram_tensor("wc_bf", [H, C], bft, kind="ExternalInput")
    bc_dr = nc.dram_tensor("bc8", [C, 1], f32, kind="ExternalInput")
    out_lgT = nc.dram_tensor("out_lgT", [C, NP], f32, kind="ExternalOutput")

    S2_np, ones_np, onesrow_np, sel4_np = _build_consts()
    S2_dr = nc.inline_tensor(S2_np, "s2_const")
    ones_dr = nc.inline_tensor(ones_np, "ones_const")
    onesrow_dr = nc.inline_tensor(onesrow_np, "onesrow_const")
    sel4_dr = nc.inline_tensor(sel4_np, "sel4_const")

    # collective buffers
    hs_cc_in = nc.dram_tensor("hs_cc_in", [E, PL], bft)
    hs_cc_out = nc.dram_tensor("hs_cc_out", [E, PL], bft)
    ts_cc_in = nc.dram_tensor("ts_cc_in", [E, PL], bft)
    ts_cc_o1 = nc.dram_tensor("ts_cc_o1", [NCORE, E // 2, PL], bft, addr_space="Shared")
    ts_cc_o2 = nc.dram_tensor("ts_cc_o2", [NCORE, E // 2, PL], bft, addr_space="Shared")
    lg_cc_in = nc.dram_tensor("lg_cc_in", [NCHUNK, C, CW], f32)
    lg_cc_out = nc.dram_tensor("lg_cc_out", [NCHUNK, C, CW], f32, addr_space="Shared")
    groups = [list(range(NCORE))]

    with tile.TileContext(nc) as tc:
        with (
            tc.tile_pool(name="gpool", bufs=1) as gpool,
            tc.tile_pool(name="persist", bufs=1) as persist,
        ):
            # ---------- whole-kernel-lifetime weights / constants ----------
            wp_sb = gpool.tile([128, NKT, H], bft)
            wc_sb = gpool.tile([128, 6, C], bft)
            bc_sb = gpool.tile([C, 1], f32)
            sel4_sb = gpool.tile([128, 8, 128], bft)
            ts_all = gpool.tile([128, NU, NCORE, PL], bft)
            hs_all = gpool.tile([128, NU, NCORE, PL], bft)

            hs_sb = persist.tile([128, 6, PL], bft)
            ts_sb = persist.tile([128, 6, PL], bft)

            with (
                tc.tile_pool(name="p1", bufs=1) as p1,
                tc.tile_pool(name="ps1", bufs=2, space="PSUM") as ps1,
            ):
                # ---------- DMA schedule ----------
                # sync queue: ment + e_att (critical), then the big Wp load
                ment_sb = p1.tile([128, H], f32)
                nc.sync.dma_start(out=ment_sb, in_=ment_dr[:])
                eaT = p1.tile([128, 4, 30, NH], bft)
                for lt in range(4):
                    nc.sync.dma_start(out=eaT[:, lt, :, :],
                                      in_=eat_dr[lt * 128 : (lt + 1) * 128])
                for q in range(4):
                    r0 = q * (KST // 4)
                    nc.sync.dma_start(
                        out=wp_sb[:, q * (NKT // 4) : (q + 1) * (NKT // 4), :],
                        in_=wp_dr[r0 : r0 + KST // 4].rearrange("(a p) h -> p a h", p=128),
                    )
                nc.sync.dma_start(out=wc_sb, in_=wc_dr[:].rearrange("(a p) c -> p a c", p=128))

                # gpsimd queue: small consts then phase-1 h-side weights
                nc.gpsimd.dma_start(out=bc_sb, in_=bc_dr[:])
                bh_sb = p1.tile([128, 6], f32)
                nc.gpsimd.dma_start(out=bh_sb, in_=bh_dr[:])
                bt_sb = p1.tile([128, 6], f32)
                nc.gpsimd.dma_start(out=bt_sb, in_=bt_dr[:])
                S2_sb = p1.tile([128, 30], bft)
                nc.gpsimd.dma_start(out=S2_sb, in_=S2_dr[:])
                nc.gpsimd.dma_start(out=sel4_sb, in_=sel4_dr[:])
                ones_sb = p1.tile([128, 1], bft)
                nc.gpsimd.dma_start(out=ones_sb, in_=ones_dr[:])
                onesrow_sb = p1.tile([1, 128], f32)
                nc.gpsimd.dma_start(out=onesrow_sb, in_=onesrow_dr[:])
                wh1_sb = p1.tile([128, 6, E], bft)
                nc.gpsimd.dma_start(out=wh1_sb, in_=wh1_dr[:].rearrange("(a p) e -> p a e", p=128))
                wt1_sb = p1.tile([128, 6, E], bft)
                nc.gpsimd.dma_start(out=wt1_sb, in_=wt1_dr[:].rearrange("(a p) e -> p a e", p=128))

                # scalar queue: seq + exp/ln first, then the extractor weights
                seq_sb = p1.tile([128, 4, H], bft)
                nc.scalar.dma_start(out=seq_sb, in_=seq_dr[:].rearrange("(a p) h -> p a h", p=128))
                nc.vector.memset(hs_all[:].rearrange("p a b c -> p (a b c)"), 0.0)
                exp_g = p1.tile([128, H], bft)
                nc.scalar.activation(out=exp_g, in_=ment_sb[:], func=AF.Exp)

                # eeT[h, ent] = ln(sum_m exp(ment)) via selector matmul
                eeT = p1.tile([128, 6, 30], bft)
                for ht in range(6):
                    tr = ps1.tile([128, 30], f32, tag="sm1", bufs=2)
                    nc.tensor.matmul(tr, lhsT=exp_g[:, 128 * ht : 128 * (ht + 1)],
                                     rhs=S2_sb[:], start=True, stop=True)
                    nc.scalar.activation(out=eeT[:, ht, :], in_=tr, func=AF.Ln)

                wh2_sb = p1.tile([128, 6, E], bft)
                nc.scalar.dma_start(out=wh2_sb, in_=wh2_dr[:].rearrange("(a p) e -> p a e", p=128))
                wt2_sb = p1.tile([128, 6, E], bft)
                nc.scalar.dma_start(out=wt2_sb, in_=wt2_dr[:].rearrange("(a p) e -> p a e", p=128))

                # ---------- pair attention products (unnormalized) ----------
                # (emitted before the hp/tp copies so the vector queue starts
                # on the critical path as soon as e_att arrives)
                ht_raw = p1.tile([128, 4, PL], bft)
                sum_ps = ps1.tile([1, PL], f32, tag="lsum", bufs=1)
                with nc.allow_low_precision("bf16 pair-product reduce; normalization is scale-invariant"):
                    for lt in range(4):
                        prod = p1.tile([128, IPC, NE, NH], bft, tag="prod", bufs=2)
                        in0 = eaT[:, lt, 24:30, :].unsqueeze(2).broadcast_to([128, IPC, NE, NH])
                        in1 = eaT[:, lt, 0:24, :].unsqueeze(1).broadcast_to([128, IPC, NE, NH])
                        nc.vector.tensor_mul(out=prod, in0=in0, in1=in1)
                        nc.vector.tensor_reduce(
                            out=ht_raw[:, lt, :],
                            in_=prod[:].rearrange("p a b h -> p (a b) h"),
                            axis=mybir.AxisListType.X, op=ALU.add)
                        nc.vector.tensor_scalar_max(
                            out=ht_raw[:, lt, :], in0=ht_raw[:, lt, :], scalar1=0.0)

                # hpartT [E'(perm), own-i 6] (tpartT comes after the hs
                # extractor so its 36 tiny matmuls don't delay the AllToAll)
                hpT = p1.tile([128, 6, IPC], bft)
                tpT = p1.tile([128, 6, NE], bft)
                for Et in range(6):
                    hp = ps1.tile([128, IPC], f32, tag="sm1", bufs=2)
                    for ht in range(6):
                        nc.tensor.matmul(
                            hp, lhsT=wh1_sb[:, ht, 128 * Et : 128 * (Et + 1)],
                            rhs=eeT[:, ht, 24:30], start=(ht == 0), stop=(ht == 5))
                    nc.scalar.copy(out=hpT[:, Et, :], in_=hp)

                for lt in range(4):
                    nc.tensor.matmul(sum_ps, lhsT=ones_sb[:], rhs=ht_raw[:, lt, :],
                                     start=(lt == 0), stop=(lt == 3))
                denom = p1.tile([1, PL], f32)
                nc.vector.tensor_scalar_add(out=denom, in0=sum_ps, scalar1=1e-10)
                recip = p1.tile([1, PL], f32)
                nc.vector.reciprocal(out=recip, in_=denom)
                rep_ps = ps1.tile([128, PL], f32, tag="acc", bufs=2)
                nc.tensor.matmul(rep_ps, lhsT=onesrow_sb[:], rhs=recip[:], start=True, stop=True)
                recip_rep = p1.tile([128, PL], f32)
                nc.vector.tensor_copy(out=recip_rep, in_=rep_ps)

                # ---------- rs^T = seq^T @ ht_raw, scaled by 1/sum on copy-out ----------
                rsT = p1.tile([128, 6, PL], bft)
                for ht in range(6):
                    rp = ps1.tile([128, PL], f32, tag="acc", bufs=2)
                    for lt in range(4):
                        nc.tensor.matmul(rp, lhsT=seq_sb[:, lt, 128 * ht : 128 * (ht + 1)],
                                         rhs=ht_raw[:, lt, :], start=(lt == 0), stop=(lt == 3))
                    nc.vector.tensor_mul(out=rsT[:, ht, :], in0=rp, in1=recip_rep)

                # ---------- extractors, interleaved in E-halves so the CC
                # stream runs A2A1 -> AG1 -> A2A2 -> AG2 and chunk 0 can start
                # right after AG1 ----------
                def hs_half(h0):
                    for Et in range(h0, h0 + 3):
                        hp_b = hpT[:, Et, :].unsqueeze(2).broadcast_to([128, IPC, NE])
                        ep = ps1.tile([128, PL], f32, tag="acc", bufs=2)
                        for ht in range(6):
                            nc.tensor.matmul(ep, lhsT=wh2_sb[:, ht, 128 * Et : 128 * (Et + 1)],
                                             rhs=rsT[:, ht, :], start=(ht == 0), stop=(ht == 5))
                        nc.vector.tensor_add(out=ep[:].rearrange("p (a b) -> p a b", a=IPC),
                                             in0=ep[:].rearrange("p (a b) -> p a b", a=IPC), in1=hp_b)
                        nc.scalar.activation(out=hs_sb[:, Et, :], in_=ep, func=AF.Tanh,
                                             bias=bh_sb[:, Et : Et + 1])
                    sl = slice(h0 * 128, (h0 + 3) * 128)
                    nc.scalar.dma_start(
                        out=hs_cc_in[sl].rearrange("(a p) c -> p a c", p=128),
                        in_=hs_sb[:, h0 : h0 + 3, :])
                    nc.gpsimd.collective_compute(
                        "AllToAll", ALU.bypass, replica_groups=groups,
                        ins=[hs_cc_in[sl].opt()], outs=[hs_cc_out[sl].opt()])

                def ts_half(h0, out_cc):
                    for Et in range(h0, h0 + 3):
                        tp = ps1.tile([128, NE], f32, tag="sm1", bufs=2)
                        for ht in range(6):
                            nc.tensor.matmul(
                                tp, lhsT=wt1_sb[:, ht, 128 * Et : 128 * (Et + 1)],
                                rhs=eeT[:, ht, 0:24], start=(ht == 0), stop=(ht == 5))
                        nc.scalar.copy(out=tpT[:, Et, :], in_=tp)
                    for Et in range(h0, h0 + 3):
                        tp_b = tpT[:, Et, :].unsqueeze(1).broadcast_to([128, IPC, NE])
                        ep2 = ps1.tile([128, PL], f32, tag="acc", bufs=2)
                        for ht in range(6):
                            nc.tensor.matmul(ep2, lhsT=wt2_sb[:, ht, 128 * Et : 128 * (Et + 1)],
                                             rhs=rsT[:, ht, :], start=(ht == 0), stop=(ht == 5))
                        nc.vector.tensor_add(out=ep2[:].rearrange("p (a b) -> p a b", a=IPC),
                                             in0=ep2[:].rearrange("p (a b) -> p a b", a=IPC), in1=tp_b)
                        nc.scalar.activation(out=ts_sb[:, Et, :], in_=ep2, func=AF.Tanh,
                                             bias=bt_sb[:, Et : Et + 1])
                    sl = slice(h0 * 128, (h0 + 3) * 128)
                    nc.scalar.dma_start(
                        out=ts_cc_in[sl].rearrange("(a p) c -> p a c", p=128),
                        in_=ts_sb[:, h0 : h0 + 3, :])
                    nc.gpsimd.collective_compute(
                        "AllGather", ALU.bypass, replica_groups=groups,
                        ins=[ts_cc_in[sl].opt()], outs=[out_cc[:].opt()])

                hs_half(0)
                ts_half(0, ts_cc_o1)
                hs_half(3)
                ts_half(3, ts_cc_o2)

                # redistributed hs resident in SBUF: partition kb*8+s holds, per
                # (u, d): hs value for (k=2u+kb, s) of source-core d's pairs
                for uh in range(2):
                    for d in range(NCORE):
                        for kb in range(2):
                            nc.gpsimd.dma_start(
                                out=hs_all[8 * kb : 8 * (kb + 1), 3 * uh : 3 * uh + 3, d, :],
                                in_=bass.AP(
                                    tensor=hs_cc_out,
                                    offset=(uh * 384 + d * 48 + kb * 24) * PL,
                                    ap=[[3 * PL, 8], [PL, 3], [1, PL]],
                                ),
                            )
                # ts^T of all pairs resident in SBUF: ts_all[p, u, d, :] =
                # ts row 128u+p of source-core d (partition p<64 is (k=2u, t=p),
                # p>=64 is (k=2u+1, t=p-64))
                for u in range(NU):
                    src, uo = (ts_cc_o1, u) if u < 3 else (ts_cc_o2, u - 3)
                    nc.sync.dma_start(
                        out=ts_all[:, u, :, :],
                        in_=bass.AP(
                            tensor=src, offset=uo * 128 * PL,
                            ap=[[PL, 128], [(E // 2) * PL, NCORE], [1, PL]],
                        ),
                    )

            # ---------- phase 2: feature + classifier over pair chunks ----------
            with (
                tc.tile_pool(name="p2", bufs=2) as p2,
                tc.tile_pool(name="ps2", bufs=1, space="PSUM") as ps2,
            ):
                # flat (chunk, tile) schedule: the sel matmul + its scalar
                # PSUM->SBUF bf16 staging copy run a fixed DEPTH iterations
                # ahead of the fps matmuls (crossing chunk boundaries), so no
                # producer-consumer semaphore hop is ever on the critical path
                work = [(c, t) for c in range(NCHUNK) for t in range(NKT)]
                DEPTH = 6
                b1s_ring = {}

                def emit_selcopy(i):
                    c, t = work[i]
                    u, j = t // 8, t % 8
                    ps = ps2.tile([128, CW], f32, tag="b1ps", bufs=2)
                    nc.tensor.matmul(
                        ps, lhsT=sel4_sb[:, j, :],
                        rhs=hs_all[:, u, 2 * c : 2 * c + 2, :].rearrange(
                            "p a b -> p (a b)"),
                        start=True, stop=True)
                    b1s = p2.tile([128, CW], bft, tag="b1s", bufs=DEPTH + 10)
                    nc.scalar.copy(out=b1s, in_=ps)
                    b1s_ring[i] = b1s

                for i in range(DEPTH):
                    emit_selcopy(i)

                fps = []
                for i, (ck, kt) in enumerate(work):
                    if kt == 0:
                        fps = []
                        for h in range(6):
                            fpt = ps2.tile([128, CW], f32, tag=f"feat{h}", bufs=1,
                                           name=f"fps{h}")
                            fps.append(fpt)
                    u = kt // 8
                    bl = p2.tile([128, CW], bft, tag="bl", bufs=4)
                    nc.vector.tensor_mul(
                        out=bl, in0=b1s_ring.pop(i),
                        in1=ts_all[:, u, 2 * ck : 2 * ck + 2, :].rearrange(
                            "p a b -> p (a b)"))
                    for h in range(6):
                        nc.tensor.matmul(
                            fps[h], lhsT=wp_sb[:, kt, 128 * h : 128 * (h + 1)],
                            rhs=bl, start=(kt == 0), stop=(kt == NKT - 1))
                    if i + DEPTH < len(work):
                        emit_selcopy(i + DEPTH)
                    if kt != NKT - 1:
                        continue

                    # classifier
                    lgp = ps2.tile([C, CW], f32, tag="b1ps", bufs=2)
                    for h in range(6):
                        fT = p2.tile([128, CW], bft, tag="fT", bufs=3)
                        if h % 2 == 0:
                            nc.scalar.copy(out=fT, in_=fps[h])
                        else:
                            nc.vector.tensor_copy(out=fT, in_=fps[h])
                        nc.tensor.matmul(lgp, lhsT=wc_sb[:, h, :], rhs=fT,
                                         start=(h == 0), stop=(h == 5))

                    # bias (bc/8 per core) + self-pair mask BEFORE the AllReduce
                    lgs = p2.tile([C, CW], f32, tag="lgs", bufs=2)
                    nc.vector.tensor_scalar_add(out=lgs, in0=lgp, scalar1=bc_sb[:])
                    for col in _self_cols(ck):
                        nc.vector.memset(lgs[:, col : col + 1], 0.0)
                    nc.scalar.dma_start(out=lg_cc_in[ck, :, :], in_=lgs)

                    nc.gpsimd.collective_compute(
                        "AllReduce", ALU.add, replica_groups=groups,
                        ins=[lg_cc_in[ck, :, :].opt()],
                        outs=[lg_cc_out[ck, :, :].opt()])
                    if ck >= 1:
                        nc.scalar.dma_start(
                            out=out_lgT[:, (ck - 1) * CW : ck * CW],
                            in_=lg_cc_out[ck - 1, :, :])
                nc.scalar.dma_start(
                    out=out_lgT[:, (NCHUNK - 1) * CW :],
                    in_=lg_cc_out[NCHUNK - 1, :, :])

    if not nc.is_finalized():
        nc.finalize()
    return nc


_NC_CACHE = None


def kernel(**inputs):
    global _NC_CACHE
    from concourse.bass_utils import run_bass_kernel_spmd

    if _NC_CACHE is None:
        _NC_CACHE = build_bass()
    in_maps = _host_prep(inputs)
    res = run_bass_kernel_spmd(_NC_CACHE, in_maps, core_ids=list(range(NCORE)))
    kernel.last_results = res
    out = res.results[0]["out_lgT"]  # [97, 1152]
    return np.ascontiguousarray(out.T).astype(np.float32)
